# revision 38
# baseline (speedup 1.0000x reference)
"""Trainium2 Bass kernel for nn_AttentionOperation (sparse_attention).

Computation (per the reference):
    sim  = QK^T                  [N,H,L,L]
    sim  = BN_heads(sim)         (stats over b,l,m per head)
    attn = softmax(sim, -1)
    rv   = attn @ V^T            [N,H,C,L] -> [N, H*C, L]
    rv   = BN_channels(rv)       (stats over b,l per channel)
    out  = gelu_exact(rv)

Sharding: one head per NeuronCore (H=8, n_cores=8).  Both BatchNorms are
then fully core-local (sim-BN stats are per head; val-BN channels
h*64..(h+1)*64-1 belong exactly to head h), so there is no communication.

Device-side tricks:
  * BN1 mean/bias shift cancels inside the softmax, so only
    g = w_h * rsqrt(var + eps) is needed.  var comes from tiny Gram
    matmuls: sum(sim^2) = sum_b tr((K Kt)(Q Qt)), sum(sim) = sum_b
    ksum_b . qsum_b.  K and Q chunks are stacked into one [m,129]
    operand so one matmul per (batch, m-chunk) yields KK, QQ, ksum,
    qsum together; the QQ block is realigned with a SBUF->SBUF DMA.
  * matmul operands are fp16 (fp32 matmul = 2 half-rate passes on PE).
    PSUM accumulation stays fp32; per-element error ~2^-11 averages out
    in the BN statistics.
  * softmax denominator comes free from a ones-row appended to V^T.
  * rsqrt is a DVE-only quake-seed Newton iteration, and 1/den uses the
    custom-DVE reciprocal_approx_fast, so ScalarE needs only the Exp
    and Gelu table sets (2 loads total).
  * BN2 affine is folded into the Gelu activation's scale/bias operands.
"""

import numpy as np

N, H, D, L = 4, 8, 64, 1024
C = 64
NCH = L // 128  # m-chunks of 128
EPS = 1e-3
CNT = float(N * L * L)  # elements per head for sim BN stats

_CACHE = {}


def _build_nc():
    import concourse.bacc as bacc
    import concourse.tile as tile
    import concourse.mybir as mybir

    f32 = mybir.dt.float32
    f16 = mybir.dt.float16
    i32 = mybir.dt.int32
    AF = mybir.ActivationFunctionType
    ALU = mybir.AluOpType

    nc = bacc.Bacc("TRN2", target_bir_lowering=False, debug=False)

    q2_d = nc.dram_tensor("q2", [128, 2, L], f16, kind="ExternalInput")
    k2_d = nc.dram_tensor("k2", [128, 2, L], f16, kind="ExternalInput")
    kqo_d = nc.dram_tensor("kqo", [128, N, NCH, 129], f16,
                           kind="ExternalInput")
    vo_d = nc.dram_tensor("vo", [128, N, NCH, 65], f16, kind="ExternalInput")
    id_d = nc.dram_tensor("ident", [128, 64], f32, kind="ExternalInput")
    ws_d = nc.dram_tensor("ws", [1, 1], f32, kind="ExternalInput")
    wv_d = nc.dram_tensor("wv", [64, 1], f32, kind="ExternalInput")
    bv_d = nc.dram_tensor("bv", [64, 1], f32, kind="ExternalInput")
    out_d = nc.dram_tensor("out", [N, 64, L], f32, kind="ExternalOutput")

    with tile.TileContext(nc) as tc:
        with (
            tc.tile_pool(name="cst", bufs=1) as cst,
            tc.tile_pool(name="sm", bufs=1) as sm,
            tc.tile_pool(name="exp", bufs=16) as epool,
            tc.tile_pool(name="rvp", bufs=4) as rvp,
            tc.tile_pool(name="outp", bufs=4) as outp,
            tc.tile_pool(name="ps", bufs=1, space="PSUM") as psp,
        ):
            # ---- input DMAs (gram inputs first: they gate g -> every exp)
            # one big transfer: a single dma_start fans out over all 16
            # SDMA slots with 8KB descriptors, ~4x the per-ring bandwidth
            # of four concurrent small transfers
            kqo_sb = cst.tile([128, N, NCH, 129], f16)
            nc.sync.dma_start(kqo_sb[:], kqo_d.ap())
            q2_sb = cst.tile([128, 2, L], f16)
            k2_sb = cst.tile([128, 2, L], f16)
            for p in range(2):
                nc.sync.dma_start(q2_sb[:, p], q2_d.ap()[:, p])
                nc.sync.dma_start(k2_sb[:, p], k2_d.ap()[:, p])
            ws_sb = cst.tile([1, 1], f32)
            nc.sync.dma_start(ws_sb[:], ws_d.ap())
            wv_sb = cst.tile([64, 1], f32)
            nc.sync.dma_start(wv_sb[:], wv_d.ap())
            bv_sb = cst.tile([64, 1], f32)
            nc.sync.dma_start(bv_sb[:], bv_d.ap())
            id_sb = cst.tile([128, 64], f32)
            nc.sync.dma_start(id_sb[:], id_d.ap())
            ones128 = cst.tile([1, 128], f32)
            nc.vector.memset(ones128[:], 1.0)
            ones64 = cst.tile([64, 1], f32)
            nc.vector.memset(ones64[:], 1.0)
            # dummy exp so the ACT exp-table load happens off the critical
            # path (otherwise it lands right before the first real exp)
            warm_sb = sm.tile([1, 1], f32, tag="warm", bufs=1)
            nc.scalar.activation(warm_sb[:], ones128[0:1, 0:1], AF.Exp)

            # ---- BN1 stats: one stacked gram matmul per (batch, chunk) ----
            # G[b] = [k|q|1]^T [k|q|1]:  KK = G[0:64,0:64],
            # QQ = G[64:128,64:128], ksum = G[0:64,128], qsum = G[64:128,128]
            parts = cst.tile([64, 2, N], f32)
            qparts = parts[:, 0, :]
            sparts = parts[:, 1, :]
            for b in range(N):
                gps = psp.tile([128, 129], f32, tag="av", bufs=2,
                               name=f"gram_ps_{b}")
                for c in range(NCH):
                    nc.tensor.matmul(
                        gps[:], kqo_sb[:, b, c, 0:128], kqo_sb[:, b, c, :],
                        start=(c == 0), stop=(c == NCH - 1))
                # realign the QQ block onto partitions 0-63 via a tiny
                # identity matmul (a DMA here queues behind the input
                # transfers and stalls the g-chain for ~6us).  The copies
                # run on the otherwise-idle ScalarE; the products read the
                # gram PSUM directly.
                qsrc_sb = sm.tile([128, 65], f32, tag="gk", bufs=2,
                                  name=f"qsrc_sb_{b}")
                nc.scalar.copy(qsrc_sb[64:128, :], gps[64:128, 64:129])
                qq_ps = psp.tile([64, 65], f32, tag="av", bufs=2,
                                 name=f"qq_ps_{b}")
                nc.tensor.matmul(qq_ps[:], id_sb[64:128, :],
                                 qsrc_sb[64:128, :], start=True, stop=True)
                qq_sb = sm.tile([64, 65], f32, tag="gq", bufs=2,
                                name=f"qq_sb_{b}")
                nc.scalar.copy(qq_sb[:], qq_ps[:])
                pscr = sm.tile([64, 64], f32, tag="pscr", bufs=2,
                               name=f"pscr_{b}")
                nc.vector.tensor_tensor(
                    out=pscr[:], in0=gps[0:64, 0:64], in1=qq_sb[:, 0:64],
                    op=ALU.mult)
                nc.vector.tensor_reduce(
                    out=qparts[:, b:b + 1], in_=pscr[:],
                    axis=mybir.AxisListType.X, op=ALU.add)
                nc.vector.tensor_tensor(
                    out=sparts[:, b:b + 1], in0=gps[0:64, 128:129],
                    in1=qq_sb[:, 64:65], op=ALU.mult)

            # hoisted first QK chunks: keeps PE busy while the g-chain
            # (DVE scalar ops) runs, and has sim ready for the first exps
            def emit_qk(pair, c):
                sims = []
                for b_in in range(2):
                    b = 2 * pair + b_in
                    r0 = 64 * b_in
                    sim_ps = psp.tile([128, L], f32, tag="sim", bufs=3,
                                      name=f"sim_ps_{b}_{c}")
                    for half in range(2):
                        nc.tensor.matmul(
                            sim_ps[:, 512 * half:512 * (half + 1)],
                            k2_sb[r0:r0 + 64, pair, 128 * c:128 * (c + 1)],
                            q2_sb[r0:r0 + 64, pair,
                                  512 * half:512 * (half + 1)],
                            start=True, stop=True)
                    sims.append(sim_ps)
                return sims

            pre_sims = [emit_qk(0, 0)]

            qsp = sm.tile([64, 2], f32, tag="qs", bufs=1)
            nc.vector.tensor_reduce(out=qsp[:], in_=parts[:],
                                    axis=mybir.AxisListType.X, op=ALU.add)
            # partition-sum via PE: out [1,2] = [sum(sim^2), sum(sim)]
            scps = psp.tile([1, 2], f32, tag="av", bufs=2)
            nc.tensor.matmul(scps[:], ones64[:], qsp[:], start=True,
                             stop=True)
            qs2 = sm.tile([1, 2], f32, tag="qs2", bufs=1)
            nc.vector.tensor_copy(qs2[:], scps[:])

            # DVE-only rsqrt(x + eps): quake seed + 3 Newton iterations.
            def dve_rsqrt(dst_ap, x_ap, p, pref):
                xe = sm.tile([p, 1], f32, tag=f"{pref}xe", bufs=1,
                             name=f"{pref}_xe")
                nc.vector.tensor_scalar_add(xe[:], x_ap, EPS)
                sh = sm.tile([p, 1], i32, tag=f"{pref}sh", bufs=1,
                             name=f"{pref}_sh")
                nc.vector.tensor_scalar(
                    out=sh[:], in0=xe[:].bitcast(i32), scalar1=1,
                    scalar2=None, op0=ALU.arith_shift_right)
                magic = sm.tile([p, 1], i32, tag=f"{pref}mg", bufs=1,
                                name=f"{pref}_mg")
                nc.vector.memset(magic[:], 0x5F3759DF)
                y = sm.tile([p, 1], f32, tag=f"{pref}y", bufs=1,
                            name=f"{pref}_y")
                nc.vector.tensor_tensor(out=y[:].bitcast(i32), in0=magic[:],
                                        in1=sh[:], op=ALU.subtract)
                t = sm.tile([p, 1], f32, tag=f"{pref}t", bufs=1,
                            name=f"{pref}_t")
                n_it = 2  # seed err 3.4% -> 1.7e-3 -> 4e-6: plenty here
                for it in range(n_it):
                    nc.vector.tensor_tensor(out=t[:], in0=y[:], in1=y[:],
                                            op=ALU.mult)
                    nc.vector.scalar_tensor_tensor(
                        out=t[:], in0=t[:], scalar=-0.5, in1=xe[:],
                        op0=ALU.mult, op1=ALU.mult)
                    nc.vector.scalar_tensor_tensor(
                        out=(dst_ap if it == n_it - 1 else y[:]), in0=t[:],
                        scalar=1.5, in1=y[:], op0=ALU.add, op1=ALU.mult)

            # var = E[x^2] - E[x]^2 ; g = w_h * rsqrt(var + eps)
            eq_t = sm.tile([1, 1], f32, tag="sc1", bufs=1)
            nc.vector.tensor_scalar_mul(eq_t[:], qs2[:, 0:1], 1.0 / CNT)
            m2_t = sm.tile([1, 1], f32, tag="sc2", bufs=1)
            nc.vector.scalar_tensor_tensor(
                out=m2_t[:], in0=qs2[:, 1:2], scalar=1.0 / (CNT * CNT),
                in1=qs2[:, 1:2], op0=ALU.mult, op1=ALU.mult)
            var_t = sm.tile([1, 1], f32, tag="sc3", bufs=1)
            nc.vector.tensor_tensor(out=var_t[:], in0=eq_t[:], in1=m2_t[:],
                                    op=ALU.subtract)
            rs_t = sm.tile([1, 1], f32, tag="sc5", bufs=1)
            dve_rsqrt(rs_t[:], var_t[:], 1, "g")
            g_t = sm.tile([1, 1], f32, tag="sc6", bufs=1)
            nc.vector.tensor_tensor(out=g_t[:], in0=rs_t[:], in1=ws_sb[:],
                                    op=ALU.mult)
            # broadcast g to all 128 partitions via a tiny PE outer product
            gb_ps = psp.tile([128, 1], f32, tag="av", bufs=2)
            nc.tensor.matmul(gb_ps[:], ones128[:], g_t[:], start=True,
                             stop=True)
            g128 = cst.tile([128, 1], f32)
            nc.vector.tensor_copy(g128[:], gb_ps[:])

            vo_sb = cst.tile([128, N, NCH, 65], f16)
            for b in range(N):
                nc.sync.dma_start(vo_sb[:, b], vo_d.ap()[:, b])

            # ---- main attention pipeline ----
            exp_tiles = [[None] * NCH for _ in range(N)]
            rv_tiles = []
            stats = cst.tile([64, 2 * N, 6], f32)

            def emit_exp(pair, c, sims):
                for b_in in range(2):
                    b = 2 * pair + b_in
                    ex = epool.tile([128, L], f16, tag="exp", bufs=16,
                                    name=f"exp_{b}_{c}")
                    nc.scalar.activation(ex[:], sims[b_in][:], AF.Exp,
                                         scale=g128[:, 0:1])
                    exp_tiles[b][c] = ex

            for pair in range(2):
                for c in range(NCH):
                    if pair == 0 and c < 1:
                        emit_exp(pair, c, pre_sims[c])
                    else:
                        emit_exp(pair, c, emit_qk(pair, c))

                for b_in in range(2):
                    b = 2 * pair + b_in
                    rcp_sb = sm.tile([1, L], f32, tag="rcp", bufs=2,
                                     name=f"rcp_{b}")
                    rbc_sb = sm.tile([64, L], f32, tag="rbc", bufs=2,
                                     name=f"rbc_{b}")
                    rv_sb = rvp.tile([64, L], f32, tag="rv", bufs=4,
                                     name=f"rv_{b}")
                    for half in range(2):
                        hs = slice(512 * half, 512 * (half + 1))
                        av_ps = psp.tile([65, 512], f32, tag="av", bufs=2,
                                         name=f"av_ps_{b}_{half}")
                        for c in range(NCH):
                            nc.tensor.matmul(
                                av_ps[:], vo_sb[:, b, c, :],
                                exp_tiles[b][c][:, hs],
                                start=(c == 0), stop=(c == NCH - 1))
                        # den row to partition 0 (custom-DVE recip needs a
                        # base-0 SBUF operand), then a full-tile copy --
                        # same DVE cost as a 1-row copy (free-dim-bound) --
                        # to release the PSUM slot for the next batch's AV
                        den_sb = sm.tile([1, 512], f32, tag="den", bufs=4,
                                         name=f"den_{b}_{half}")
                        nc.vector.tensor_copy(den_sb[:], av_ps[64:65, :])
                        av_sb = sm.tile([65, 512], f32, tag="avs", bufs=4,
                                        name=f"av_sb_{b}_{half}")
                        nc.vector.tensor_copy(av_sb[:], av_ps[:])
                        nc.vector.reciprocal_approx_fast(
                            out=rcp_sb[0:1, hs], in_=den_sb[:])
                        nc.gpsimd.partition_broadcast(
                            rbc_sb[:, hs], rcp_sb[0:1, hs], channels=64)
                        nc.vector.tensor_tensor(
                            out=rv_sb[:, hs], in0=av_sb[0:64, :],
                            in1=rbc_sb[:, hs], op=ALU.mult)
                        nc.vector.bn_stats(stats[:, 2 * b + half, :],
                                           rv_sb[:, hs])
                    rv_tiles.append(rv_sb)

            # ---- BN2 + gelu epilogue (affine folded into Gelu) ----
            mv = sm.tile([64, 2], f32, tag="mv", bufs=1)
            nc.vector.bn_aggr(mv[:], stats[:])
            rsv = sm.tile([64, 1], f32, tag="rsv", bufs=1)
            dve_rsqrt(rsv[:], mv[:, 1:2], 64, "v")
            scale_c = sm.tile([64, 1], f32, tag="sclc", bufs=1)
            nc.vector.tensor_tensor(out=scale_c[:], in0=rsv[:], in1=wv_sb[:],
                                    op=ALU.mult)
            mt = sm.tile([64, 1], f32, tag="mt", bufs=1)
            nc.vector.tensor_tensor(out=mt[:], in0=mv[:, 0:1], in1=scale_c[:],
                                    op=ALU.mult)
            bias_c = sm.tile([64, 1], f32, tag="bsc", bufs=1)
            nc.vector.tensor_tensor(out=bias_c[:], in0=bv_sb[:], in1=mt[:],
                                    op=ALU.subtract)

            for b in range(N):
                osb = outp.tile([64, L], f32, tag="osb", bufs=4,
                                name=f"osb_{b}")
                nc.scalar.activation(osb[:], rv_tiles[b][:], AF.Gelu,
                                     bias=bias_c[:, 0:1],
                                     scale=scale_c[:, 0:1])
                nc.sync.dma_start(out_d.ap()[b], osb[:])

    nc.compile()
    return nc


def _host_inputs(query, key, value, bn_sim_weight, bn_sim_bias,
                 bn_val_weight, bn_val_bias, h):
    """Build the per-core (per-head) input map, with host-side layout prep."""
    f32 = np.float32
    f16 = np.float16
    qh = np.asarray(query[:, h], dtype=f32)   # [4, 64, 1024]
    kh = np.asarray(key[:, h], dtype=f32)
    vh = np.asarray(value[:, h], dtype=f32)

    def pack_pairs(x):
        # [4, 64, L] -> [128, 2, L]; row b_in*64+d, col (pair, l)
        return np.ascontiguousarray(
            x.reshape(2, 2, 64, L).transpose(1, 2, 0, 3).reshape(128, 2, L)
            .astype(f16))

    def chunked_t(x):
        # [4, 64, L] -> [128(m), 4(b), 8(chunk), 64]
        return x.transpose(2, 0, 1).reshape(NCH, 128, N, 64).transpose(
            1, 2, 0, 3)

    kq = np.empty((128, N, NCH, 129), dtype=f16)
    kq[..., 0:64] = chunked_t(kh).astype(f16)
    kq[..., 64:128] = chunked_t(qh).astype(f16)
    kq[..., 128] = 1.0

    vo = np.empty((128, N, NCH, 65), dtype=f16)
    vo[..., :64] = chunked_t(vh).astype(f16)
    vo[..., 64] = 1.0

    ident = np.zeros((128, 64), dtype=f32)
    ident[64:128] = np.eye(64, dtype=f32)

    return {
        "ident": ident,
        "q2": pack_pairs(qh),
        "k2": pack_pairs(kh),
        "kqo": np.ascontiguousarray(kq),
        "vo": np.ascontiguousarray(vo),
        "ws": np.asarray(bn_sim_weight[h], dtype=f32).reshape(1, 1),
        "wv": np.ascontiguousarray(
            np.asarray(bn_val_weight[h * 64:(h + 1) * 64], dtype=f32)
            .reshape(64, 1)),
        "bv": np.ascontiguousarray(
            np.asarray(bn_val_bias[h * 64:(h + 1) * 64], dtype=f32)
            .reshape(64, 1)),
    }


def get_nc():
    if "nc" not in _CACHE:
        _CACHE["nc"] = _build_nc()
    return _CACHE["nc"]


def make_in_maps(**inputs):
    return [_host_inputs(
        inputs["query"], inputs["key"], inputs["value"],
        inputs["bn_sim_weight"], inputs["bn_sim_bias"],
        inputs["bn_val_weight"], inputs["bn_val_bias"], h) for h in range(H)]


def kernel(**inputs):
    from concourse.bass_utils import run_bass_kernel_spmd

    nc = get_nc()
    in_maps = make_in_maps(**inputs)
    res = run_bass_kernel_spmd(nc, in_maps, core_ids=list(range(H)))
    outs = [np.asarray(res.results[i]["out"]) for i in range(H)]
    return np.ascontiguousarray(
        np.concatenate(outs, axis=1).astype(np.float32))
